# revision 26
# baseline (speedup 1.0000x reference)
# Trainium2 Bass kernel: transformer encoder block, 8-way sequence-parallel.
#
# Sharding: each of the 8 cores owns S/8 = 512 sequence rows (queries). Each
# core computes q^T / k^T / v for its own rows, the k^T and v shards are
# AllGathered (in bf16 to halve wire bytes) so every core sees the full
# keys/values, then each core runs full attention + out-proj + LN + FF + LN
# for its own 512 rows. The host only transposes/slices inputs and
# concatenates/transposes the output.
#
# On-chip data layout is "transposed world": activations live as [E, s]
# tiles (embedding on the partition axis) so every matmul uses the natural
# weight matrix as the stationary lhsT operand and chains without any
# activation transposes. Softmax runs on scores^T [s_k, s_q] tiles: exp on
# the scalar engine, and both the PV product and the softmax denominator
# come from one PE accumulation using a ones-augmented V ([v | 1], M=65).
# Attention processes head PAIRS so the two scores matmuls (K=64, base
# partitions 0/64) pack into disjoint PE row groups and run concurrently,
# and the PV matmuls of step kt-1 are emitted after the scores of step kt
# so the PE never stalls on the scalar engine's exp.
# Projection/FF matmuls use float32r (fp32 bits, 1 cycle/row at N>=256);
# the attention island (q/k/v/exp) runs in bf16.
import sys

if '/opt/trn_rl_repo' not in sys.path:
    sys.path.insert(0, '/opt/trn_rl_repo')

import numpy as np

import concourse.bacc as bacc
import concourse.tile as tile
import concourse.mybir as mybir
from concourse.bass_utils import run_bass_kernel_spmd

F32 = mybir.dt.float32
F32R = mybir.dt.float32r
BF16 = mybir.dt.bfloat16
ISLAND = BF16  # attention-island dtype
PIPELINE = True  # software-pipeline PV behind next scores
AF = mybir.ActivationFunctionType
ALU = mybir.AluOpType

E, H, DK, FF = 1024, 16, 64, 4096
EPS = 1e-5
P = 128

# Knobs test.py can flip.
TRACE = False
TRACE_KWARGS = {}
LAST_RESULT = None


def r32(ap):
    return ap.bitcast(F32R)


def build_nc(s_total=4096, n_cores=8):
    sl = s_total // n_cores      # local sequence rows per core
    kte = E // P                 # contraction tiles over E
    skt = s_total // P           # key tiles over full sequence
    ffe = FF // P                # tiles over FF dim
    nte = E // 512               # 512-wide free tiles over E
    stl = sl // P                # local sequence tiles
    assert sl <= 512

    nc = bacc.Bacc("TRN2", target_bir_lowering=False, debug=False,
                   num_devices=n_cores)

    def din(name, shape, dt=F32R):
        return nc.dram_tensor(name, shape, dt, kind="ExternalInput").ap()

    xT_d = din("xT", [E, sl])
    wq_d, wk_d, wv_d, wo_d = (din(n, [E, E]) for n in ("Wq", "Wk", "Wv", "Wo"))
    wf1_d = din("W_ff1", [E, FF])
    wf2_d = din("W_ff2", [FF, E])
    bq_d, bk_d, bv_d, bo_d = (din(n, [E]) for n in ("bq", "bk", "bv", "bo"))
    bf1_d = din("b_ff1", [FF])
    bf2_d = din("b_ff2", [E])
    g1_d, b1_d, g2_d, b2_d = (din(n, [E], F32) for n in ("g1", "b1", "g2", "b2"))
    outT_d = nc.dram_tensor("outT", [E, sl], F32R, kind="ExternalOutput").ap()

    ck_in = nc.dram_tensor("ck_in", [E, sl], ISLAND).ap()
    cv_in = nc.dram_tensor("cv_in", [sl, E], ISLAND).ap()
    ck_out = nc.dram_tensor("ck_out", [n_cores * E, sl], ISLAND,
                            addr_space="Shared").ap()
    cv_out = nc.dram_tensor("cv_out", [s_total, E], ISLAND,
                            addr_space="Shared").ap()
    rg = [list(range(n_cores))]

    with tile.TileContext(nc) as tc, \
            nc.allow_low_precision(reason="fp32r/bf16 matmul pipeline"):
        with (
            tc.tile_pool(name="const", bufs=1) as cp,
            tc.tile_pool(name="acts", bufs=1) as ap_,
            tc.tile_pool(name="stat_sb", bufs=3) as statp,
        ):
            # ---- constants ----
            xt = cp.tile([P, kte, sl], F32R, tag="xt")
            nc.sync.dma_start(xt[:], xT_d.rearrange("(kt p) s -> p kt s", p=P))
            # f32r/bf16 constants must be produced by a rounding engine op,
            # not memset: memset an f32 scratch, then DVE-copy (casts).
            ones_f32 = cp.tile([P, 512], F32, tag="ones_f32")
            nc.vector.memset(ones_f32[:], 1.0)
            ones_row = cp.tile([1, 512], F32R, tag="ones_row")
            nc.vector.tensor_copy(ones_row[:], ones_f32[0:1, 0:512])
            ones_col = cp.tile([P, 1], F32R, tag="ones_col")
            nc.vector.tensor_copy(ones_col[:], ones_f32[:, 0:1])
            eps_t = cp.tile([1, 1], F32, tag="eps")
            nc.vector.memset(eps_t[:], EPS)

            def row_tile(pool, d, n, tag):
                t = pool.tile([1, n], F32R, tag=tag)
                nc.sync.dma_start(t[:], d.unsqueeze(0))
                return t

            def col_tile(d, n, tag):
                t = cp.tile([P, n // P], F32, tag=tag)
                nc.sync.dma_start(t[:], d.rearrange("(t p) -> p t", p=P))
                return t

            g1_c = col_tile(g1_d, E, "g1")
            b1_c = col_tile(b1_d, E, "b1")
            g2_c = col_tile(g2_d, E, "g2")
            b2_c = col_tile(b2_d, E, "b2")

            # persistent activations
            y1T = ap_.tile([P, kte, sl], F32R, tag="y1T")

            def layer_norm(src, res, g_c, b_c, dst, psp, pstat, sqp):
                """dst = res + LN(src) * g + b, over the partition (E) axis.

                src is centered in place. All of src/res/dst are [P, kte, sl]
                sbuf views.
                """
                psmu = pstat.tile([1, sl], F32, tag="stat")
                for mt in range(kte):
                    nc.tensor.matmul(psmu[:], r32(ones_col[:]),
                                     r32(src[:, mt, :]),
                                     start=(mt == 0), stop=(mt == kte - 1))
                mu = statp.tile([1, sl], F32R, tag="stat_sb")
                nc.scalar.mul(mu[:], psmu[:], 1.0 / E)
                psmb = psp.tile([P, sl], F32, tag="ps")
                nc.tensor.matmul(psmb[:], r32(ones_row[0:1, 0:P]), r32(mu[:]),
                                 start=True, stop=True)
                for mt in range(kte):
                    nc.vector.tensor_tensor(src[:, mt, :], src[:, mt, :],
                                            psmb[:], ALU.subtract)
                psvar = pstat.tile([1, sl], F32, tag="stat")
                for mt in range(kte):
                    sq = sqp.tile([P, sl], F32R, tag="sq")
                    nc.vector.tensor_tensor(sq[:], src[:, mt, :],
                                            src[:, mt, :], ALU.mult)
                    nc.tensor.matmul(psvar[:], r32(ones_col[:]), r32(sq[:]),
                                     start=(mt == 0), stop=(mt == kte - 1))
                sstd = statp.tile([1, sl], F32R, tag="stat_sb")
                nc.scalar.activation(sstd[:], psvar[:], AF.Sqrt,
                                     bias=eps_t[0:1, 0:1], scale=1.0 / E)
                rstd = statp.tile([1, sl], F32R, tag="stat_sb")
                nc.vector.reciprocal(rstd[:], sstd[:])
                psrb = psp.tile([P, sl], F32, tag="ps")
                nc.tensor.matmul(psrb[:], r32(ones_row[0:1, 0:P]), r32(rstd[:]),
                                 start=True, stop=True)
                for mt in range(kte):
                    tmp = sqp.tile([P, sl], F32R, tag="sq")
                    nc.vector.tensor_tensor(tmp[:], src[:, mt, :], psrb[:],
                                            ALU.mult)
                    nc.vector.tensor_scalar(tmp[:], tmp[:],
                                            g_c[:, mt:mt + 1], b_c[:, mt:mt + 1],
                                            ALU.mult, ALU.add)
                    nc.vector.tensor_tensor(dst[:, mt, :], tmp[:],
                                            res[:, mt, :], ALU.add)

            with tc.tile_pool(name="qh", bufs=1) as qhp:
                qT = qhp.tile([P, kte, sl], ISLAND, tag="qT")
                heads = qhp.tile([P, kte, sl], F32R, tag="heads")

                # ---------------- phase 1: QKV + AllGather ----------------
                with (
                    tc.tile_pool(name="wcol", bufs=3) as wcp,
                    tc.tile_pool(name="wv_p", bufs=2) as wvp,
                    tc.tile_pool(name="kvstg", bufs=3) as stgp,
                    tc.tile_pool(name="rows1", bufs=1) as rp1,
                    tc.tile_pool(name="ps_qkv", bufs=3, space="PSUM") as psq,
                ):
                    bq_r = row_tile(rp1, bq_d, E, "bq")
                    bk_r = row_tile(rp1, bk_d, E, "bk")
                    bv_r = row_tile(rp1, bv_d, E, "bv")

                    def wcol_chunk(wd, mt):
                        w = wcp.tile([P, kte, P], F32R, tag="wcol")
                        nc.sync.dma_start(
                            w[:], wd[:, mt * P:(mt + 1) * P]
                            .rearrange("(kt p) m -> p kt m", p=P))
                        return w

                    # k^T -> ck_in (bf16) -> AllGather
                    for mt in range(kte):
                        w = wcol_chunk(wk_d, mt)
                        ps = psq.tile([P, sl], F32, tag="ps")
                        for kt in range(kte):
                            nc.tensor.matmul(ps[:], w[:, kt, :],
                                             xt[:, kt, :],
                                             start=(kt == 0), stop=False)
                        nc.tensor.matmul(ps[:], bk_r[0:1, mt * P:(mt + 1) * P],
                                         ones_row[0:1, 0:sl],
                                         start=False, stop=True)
                        stg = stgp.tile([P, sl], ISLAND, tag="kv_stage")
                        nc.vector.tensor_copy(stg[:], ps[:])
                        nc.sync.dma_start(ck_in[mt * P:(mt + 1) * P, :], stg[:])
                    nc.gpsimd.collective_compute(
                        "AllGather", ALU.bypass, replica_groups=rg,
                        ins=[ck_in.opt()], outs=[ck_out.opt()])

                    # v -> cv_in (bf16) -> AllGather
                    for nt in range(nte):
                        wv = wvp.tile([P, kte, 512], F32R, tag="wv")
                        nc.sync.dma_start(
                            wv[:], wv_d[:, nt * 512:(nt + 1) * 512]
                            .rearrange("(kt p) m -> p kt m", p=P))
                        for st in range(stl):
                            ps = psq.tile([P, 512], F32, tag="ps")
                            for kt in range(kte):
                                nc.tensor.matmul(
                                    ps[:], xt[:, kt, st * P:(st + 1) * P],
                                    wv[:, kt, :],
                                    start=(kt == 0), stop=False)
                            nc.tensor.matmul(ps[:], ones_row[0:1, 0:P],
                                             bv_r[0:1, nt * 512:(nt + 1) * 512],
                                             start=False, stop=True)
                            stg = stgp.tile([P, 512], ISLAND, tag="kv_stage")
                            nc.vector.tensor_copy(stg[:], ps[:])
                            nc.sync.dma_start(
                                cv_in[st * P:(st + 1) * P,
                                      nt * 512:(nt + 1) * 512], stg[:])
                    nc.gpsimd.collective_compute(
                        "AllGather", ALU.bypass, replica_groups=rg,
                        ins=[cv_in.opt()], outs=[cv_out.opt()])

                    # q^T -> sbuf bf16 (overlaps with the AllGathers)
                    for mt in range(kte):
                        w = wcol_chunk(wq_d, mt)
                        ps = psq.tile([P, sl], F32, tag="ps")
                        for kt in range(kte):
                            nc.tensor.matmul(ps[:], w[:, kt, :],
                                             xt[:, kt, :],
                                             start=(kt == 0), stop=False)
                        nc.tensor.matmul(ps[:], bq_r[0:1, mt * P:(mt + 1) * P],
                                         ones_row[0:1, 0:sl],
                                         start=False, stop=True)
                        nc.vector.tensor_copy(qT[:, mt, :], ps[:])

                # ---------------- phase 2: attention (bf16 island) ---------
                with (
                    tc.tile_pool(name="attn", bufs=2) as atp,
                    tc.tile_pool(name="vau", bufs=4) as vap,
                    tc.tile_pool(name="exp_p", bufs=4) as exq,
                    tc.tile_pool(name="ps_s", bufs=2, space="PSUM") as pss_p,
                    tc.tile_pool(name="ps_o", bufs=2, space="PSUM") as pso_p,
                    tc.tile_pool(name="ps_b", bufs=2, space="PSUM") as psb_p,
                ):
                    for hp in range(H // 2):
                        hA, hB = 2 * hp, 2 * hp + 1
                        # k^T head pair: rows 0:64 head A, 64:128 head B
                        kth = atp.tile([P, s_total], ISLAND, tag="kth")
                        for r in range(n_cores):
                            for sub, h in ((0, hA), (1, hB)):
                                nc.sync.dma_start(
                                    kth[sub * DK:(sub + 1) * DK,
                                        r * sl:(r + 1) * sl],
                                    ck_out[r * E + h * DK:
                                           r * E + (h + 1) * DK, :])
                        vaugs = []
                        for h in (hA, hB):
                            va = vap.tile([P, skt, DK + 1], ISLAND, tag="vaug")
                            nc.sync.dma_start(
                                va[:, :, 0:DK],
                                cv_out[:, h * DK:(h + 1) * DK]
                                .rearrange("(t p) d -> p t d", p=P))
                            nc.vector.tensor_copy(
                                va[:, :, DK:DK + 1],
                                ones_f32[:, 0:skt].unsqueeze(2))
                            vaugs.append(va)
                        vaA, vaB = vaugs

                        psoA = pso_p.tile([DK + 1, sl], F32, tag="pso")
                        psoB = pso_p.tile([DK + 1, sl], F32, tag="pso")
                        qA = qT[0:DK, hp, :]
                        qB = qT[DK:2 * DK, hp, :]
                        ex_prev = None
                        for kt in range(skt):
                            # the two scores matmuls pack into PE row groups
                            # 0-63 / 64-127 and run concurrently
                            pss = pss_p.tile([P, 2, sl], F32, tag="pss")
                            nc.tensor.matmul(pss[:, 0, :],
                                             kth[0:DK, kt * P:(kt + 1) * P],
                                             qA, start=True, stop=True)
                            nc.tensor.matmul(pss[:, 1, :],
                                             kth[DK:2 * DK, kt * P:(kt + 1) * P],
                                             qB, start=True, stop=True)
                            ex = exq.tile([P, 2, sl], ISLAND, tag="ex")
                            nc.scalar.activation(ex[:], pss[:], AF.Exp,
                                                 scale=0.125)
                            if PIPELINE:
                                if ex_prev is not None:
                                    ktp = kt - 1
                                    nc.tensor.matmul(
                                        psoA[:], vaA[:, ktp, :],
                                        ex_prev[:, 0, :],
                                        start=(ktp == 0), stop=False,
                                        skip_group_check=True)
                                    nc.tensor.matmul(
                                        psoB[:], vaB[:, ktp, :],
                                        ex_prev[:, 1, :],
                                        start=(ktp == 0), stop=False,
                                        skip_group_check=True)
                                ex_prev = ex
                            else:
                                nc.tensor.matmul(
                                    psoA[:], vaA[:, kt, :], ex[:, 0, :],
                                    start=(kt == 0), stop=(kt == skt - 1),
                                    skip_group_check=True)
                                nc.tensor.matmul(
                                    psoB[:], vaB[:, kt, :], ex[:, 1, :],
                                    start=(kt == 0), stop=(kt == skt - 1),
                                    skip_group_check=True)
                        if PIPELINE:
                            ktp = skt - 1
                            nc.tensor.matmul(psoA[:], vaA[:, ktp, :],
                                             ex_prev[:, 0, :],
                                             start=False, stop=True,
                                             skip_group_check=True)
                            nc.tensor.matmul(psoB[:], vaB[:, ktp, :],
                                             ex_prev[:, 1, :],
                                             start=False, stop=True,
                                             skip_group_check=True)

                        # normalize: heads_h = pso[0:64] / pso[64]
                        for sub, pso in ((0, psoA), (1, psoB)):
                            off = sub * DK
                            rc_t = exq.tile([1, sl], F32R, tag="recip")
                            nc.vector.reciprocal(rc_t[:], pso[DK:DK + 1, :])
                            psb = psb_p.tile([DK, sl], F32, tag="psb")
                            nc.tensor.matmul(psb[:], ones_row[0:1, 0:DK],
                                             rc_t[:], start=True, stop=True)
                            nc.vector.tensor_copy(heads[off:off + DK, hp, :],
                                                  pso[0:DK, :])
                            nc.vector.tensor_tensor(heads[off:off + DK, hp, :],
                                                    heads[off:off + DK, hp, :],
                                                    psb[:], ALU.mult)

                # ---------------- phase 3: out-proj + LN1 ------------------
                with (
                    tc.tile_pool(name="wo_p", bufs=3) as wop,
                    tc.tile_pool(name="rows3", bufs=1) as rp3,
                    tc.tile_pool(name="z_p", bufs=1) as zp,
                    tc.tile_pool(name="sq3", bufs=2) as sq3,
                    tc.tile_pool(name="ps_m3", bufs=3, space="PSUM") as psm3,
                    tc.tile_pool(name="ps_st3", bufs=2, space="PSUM") as pst3,
                ):
                    bo_r = row_tile(rp3, bo_d, E, "bo")
                    zT = zp.tile([P, kte, sl], F32R, tag="zT")
                    for mt in range(kte):
                        w = wop.tile([P, kte, P], F32R, tag="wo")
                        nc.sync.dma_start(
                            w[:], wo_d[:, mt * P:(mt + 1) * P]
                            .rearrange("(kt p) m -> p kt m", p=P))
                        ps = psm3.tile([P, sl], F32, tag="ps")
                        for kt in range(kte):
                            nc.tensor.matmul(ps[:], w[:, kt, :],
                                             heads[:, kt, :],
                                             start=(kt == 0), stop=False)
                        nc.tensor.matmul(ps[:], bo_r[0:1, mt * P:(mt + 1) * P],
                                         ones_row[0:1, 0:sl],
                                         start=False, stop=True)
                        nc.vector.tensor_copy(zT[:, mt, :], ps[:])
                    layer_norm(zT, xt, g1_c, b1_c, y1T, psm3, pst3, sq3)

            # ---------------- phases 4-6: FF + LN2 ----------------
            with (
                tc.tile_pool(name="ff", bufs=1) as ffp,
                tc.tile_pool(name="wf1_p", bufs=4) as wf1p,
                tc.tile_pool(name="wf2_p", bufs=3) as wf2p,
                tc.tile_pool(name="sq4", bufs=2) as sq4,
                tc.tile_pool(name="ps_m4", bufs=3, space="PSUM") as psm4,
                tc.tile_pool(name="ps_st4", bufs=2, space="PSUM") as pst4,
            ):
                hT = ffp.tile([P, ffe, sl], F32R, tag="hT")
                ffT = ffp.tile([P, kte, sl], F32R, tag="ffT")
                bf1_r = row_tile(ffp, bf1_d, FF, "bf1")
                bf2_r = row_tile(ffp, bf2_d, E, "bf2")
                for mt in range(ffe):
                    wt = wf1p.tile([P, kte, P], F32R, tag="wf1")
                    nc.sync.dma_start(
                        wt[:], wf1_d[:, mt * P:(mt + 1) * P]
                        .rearrange("(kt p) m -> p kt m", p=P))
                    ps = psm4.tile([P, sl], F32, tag="ps")
                    for kt in range(kte):
                        nc.tensor.matmul(ps[:], wt[:, kt, :],
                                         y1T[:, kt, :],
                                         start=(kt == 0), stop=False)
                    nc.tensor.matmul(ps[:], bf1_r[0:1, mt * P:(mt + 1) * P],
                                     ones_row[0:1, 0:sl],
                                     start=False, stop=True)
                    nc.vector.tensor_scalar_max(hT[:, mt, :], ps[:], 0.0)
                kg = 8  # kt-group size for streaming W_ff2
                for mt in range(kte):
                    ps = psm4.tile([P, sl], F32, tag="ps")
                    for g in range(ffe // kg):
                        wt2 = wf2p.tile([P, kg, P], F32R, tag="wf2")
                        nc.sync.dma_start(
                            wt2[:], wf2_d[g * kg * P:(g + 1) * kg * P,
                                          mt * P:(mt + 1) * P]
                            .rearrange("(kt p) m -> p kt m", p=P))
                        for j in range(kg):
                            kt = g * kg + j
                            nc.tensor.matmul(ps[:], wt2[:, j, :],
                                             hT[:, kt, :],
                                             start=(kt == 0), stop=False)
                    nc.tensor.matmul(ps[:], bf2_r[0:1, mt * P:(mt + 1) * P],
                                     ones_row[0:1, 0:sl],
                                     start=False, stop=True)
                    nc.vector.tensor_copy(ffT[:, mt, :], ps[:])
                layer_norm(ffT, y1T, g2_c, b2_c, ffT, psm4, pst4, sq4)
                for mt in range(kte):
                    nc.sync.dma_start(outT_d[mt * P:(mt + 1) * P, :],
                                      ffT[:, mt, :])

    nc.compile()
    return nc


_CACHE = {}


def kernel(**inputs):
    global LAST_RESULT
    inp = {k: np.ascontiguousarray(np.asarray(v, dtype=np.float32))
           for k, v in inputs.items()}
    x = inp['encoder_input']
    s_total = x.shape[0]
    n_cores = 8
    sl = s_total // n_cores

    key = (s_total, n_cores)
    if key not in _CACHE:
        _CACHE[key] = build_nc(s_total=s_total, n_cores=n_cores)
    nc = _CACHE[key]

    xT = np.ascontiguousarray(x.T)
    common = {n: inp[n] for n in
              ("Wq", "Wk", "Wv", "Wo", "W_ff1", "W_ff2",
               "bq", "bk", "bv", "bo", "b_ff1", "b_ff2",
               "g1", "b1", "g2", "b2")}
    in_maps = [{"xT": np.ascontiguousarray(xT[:, r * sl:(r + 1) * sl]), **common}
               for r in range(n_cores)]

    res = run_bass_kernel_spmd(nc, in_maps, list(range(n_cores)),
                               trace=TRACE, **TRACE_KWARGS)
    LAST_RESULT = res
    out = np.concatenate([res.results[r]["outT"] for r in range(n_cores)],
                         axis=1).T
    return np.ascontiguousarray(out)


# revision 28
# speedup vs baseline: 1.0366x; 1.0366x over previous
# Trainium2 Bass kernel: transformer encoder block, 8-way sequence-parallel.
#
# Sharding: each of the 8 cores owns S/8 = 512 sequence rows (queries). Each
# core computes q^T / k^T / v for its own rows, the k^T and v shards are
# AllGathered (in bf16 to halve wire bytes) so every core sees the full
# keys/values, then each core runs full attention + out-proj + LN + FF + LN
# for its own 512 rows. The host only transposes/slices inputs and
# concatenates/transposes the output.
#
# On-chip data layout is "transposed world": activations live as [E, s]
# tiles (embedding on the partition axis) so every matmul uses the natural
# weight matrix as the stationary lhsT operand and chains without any
# activation transposes. Softmax runs on scores^T [s_k, s_q] tiles: exp on
# the scalar engine, and both the PV product and the softmax denominator
# come from one PE accumulation using a ones-augmented V ([v | 1], M=65).
# Attention processes head PAIRS so the two scores matmuls (K=64, base
# partitions 0/64) pack into disjoint PE row groups and run concurrently,
# and the PV matmuls of step kt-1 are emitted after the scores of step kt
# so the PE never stalls on the scalar engine's exp.
# Projection/FF matmuls use float32r (fp32 bits, 1 cycle/row at N>=256);
# the attention island (q/k/v/exp) runs in bf16.
import sys

if '/opt/trn_rl_repo' not in sys.path:
    sys.path.insert(0, '/opt/trn_rl_repo')

import numpy as np
import ml_dtypes

import concourse.bacc as bacc
import concourse.tile as tile
import concourse.mybir as mybir
from concourse.bass_utils import run_bass_kernel_spmd

F32 = mybir.dt.float32
F32R = mybir.dt.float32r
BF16 = mybir.dt.bfloat16
ISLAND = BF16  # attention-island dtype
PIPELINE = True  # software-pipeline PV behind next scores
AF = mybir.ActivationFunctionType
ALU = mybir.AluOpType

E, H, DK, FF = 1024, 16, 64, 4096
EPS = 1e-5
P = 128

# Knobs test.py can flip.
TRACE = False
TRACE_KWARGS = {}
LAST_RESULT = None


def r32(ap):
    return ap.bitcast(F32R)


def build_nc(s_total=4096, n_cores=8):
    sl = s_total // n_cores      # local sequence rows per core
    kte = E // P                 # contraction tiles over E
    skt = s_total // P           # key tiles over full sequence
    ffe = FF // P                # tiles over FF dim
    nte = E // 512               # 512-wide free tiles over E
    stl = sl // P                # local sequence tiles
    assert sl <= 512

    nc = bacc.Bacc("TRN2", target_bir_lowering=False, debug=False,
                   num_devices=n_cores)

    def din(name, shape, dt=F32R):
        return nc.dram_tensor(name, shape, dt, kind="ExternalInput").ap()

    xT_d = din("xT", [E, sl])
    xTb_d = din("xTb", [E, sl], BF16)
    wq_d, wk_d, wv_d = (din(n, [E, E], BF16) for n in ("Wq", "Wk", "Wv"))
    wo_d = din("Wo", [E, E])
    wf1_d = din("W_ff1", [E, FF], BF16)
    wf2_d = din("W_ff2", [FF, E], BF16)
    bq_d, bk_d, bv_d, bo_d = (din(n, [E]) for n in ("bq", "bk", "bv", "bo"))
    bf1_d = din("b_ff1", [FF])
    bf2_d = din("b_ff2", [E])
    g1_d, b1_d, g2_d, b2_d = (din(n, [E], F32) for n in ("g1", "b1", "g2", "b2"))
    outT_d = nc.dram_tensor("outT", [E, sl], F32R, kind="ExternalOutput").ap()

    ck_in = nc.dram_tensor("ck_in", [E, sl], ISLAND).ap()
    cv_in = nc.dram_tensor("cv_in", [sl, E], ISLAND).ap()
    ck_out = nc.dram_tensor("ck_out", [n_cores * E, sl], ISLAND,
                            addr_space="Shared").ap()
    cv_out = nc.dram_tensor("cv_out", [s_total, E], ISLAND,
                            addr_space="Shared").ap()
    rg = [list(range(n_cores))]

    with tile.TileContext(nc) as tc, \
            nc.allow_low_precision(reason="fp32r/bf16 matmul pipeline"):
        with (
            tc.tile_pool(name="const", bufs=1) as cp,
            tc.tile_pool(name="acts", bufs=1) as ap_,
            tc.tile_pool(name="stat_sb", bufs=3) as statp,
        ):
            # ---- constants ----
            xt = cp.tile([P, kte, sl], F32R, tag="xt")
            nc.sync.dma_start(xt[:], xT_d.rearrange("(kt p) s -> p kt s", p=P))
            xtb = cp.tile([P, kte, sl], BF16, tag="xtb")
            nc.sync.dma_start(xtb[:], xTb_d.rearrange("(kt p) s -> p kt s", p=P))
            # f32r/bf16 constants must be produced by a rounding engine op,
            # not memset: memset an f32 scratch, then DVE-copy (casts).
            ones_f32 = cp.tile([P, 512], F32, tag="ones_f32")
            nc.vector.memset(ones_f32[:], 1.0)
            ones_row = cp.tile([1, 512], F32R, tag="ones_row")
            nc.vector.tensor_copy(ones_row[:], ones_f32[0:1, 0:512])
            ones_col = cp.tile([P, 1], F32R, tag="ones_col")
            nc.vector.tensor_copy(ones_col[:], ones_f32[:, 0:1])
            eps_t = cp.tile([1, 1], F32, tag="eps")
            nc.vector.memset(eps_t[:], EPS)

            def row_tile(pool, d, n, tag):
                t = pool.tile([1, n], F32R, tag=tag)
                nc.sync.dma_start(t[:], d.unsqueeze(0))
                return t

            def col_tile(d, n, tag):
                t = cp.tile([P, n // P], F32, tag=tag)
                nc.sync.dma_start(t[:], d.rearrange("(t p) -> p t", p=P))
                return t

            g1_c = col_tile(g1_d, E, "g1")
            b1_c = col_tile(b1_d, E, "b1")
            g2_c = col_tile(g2_d, E, "g2")
            b2_c = col_tile(b2_d, E, "b2")

            # persistent activations
            y1T = ap_.tile([P, kte, sl], F32R, tag="y1T")
            recips = ap_.tile([1, H, sl], F32R, tag="recips")

            def layer_norm(src, res, g_c, b_c, dst, psp, pstat, sqp):
                """dst = res + LN(src) * g + b, over the partition (E) axis.

                src is centered in place. All of src/res/dst are [P, kte, sl]
                sbuf views.
                """
                psmu = pstat.tile([1, sl], F32, tag="stat")
                for mt in range(kte):
                    nc.tensor.matmul(psmu[:], r32(ones_col[:]),
                                     r32(src[:, mt, :]),
                                     start=(mt == 0), stop=(mt == kte - 1))
                mu = statp.tile([1, sl], F32R, tag="stat_sb")
                nc.scalar.mul(mu[:], psmu[:], 1.0 / E)
                psmb = psp.tile([P, sl], F32, tag="ps")
                nc.tensor.matmul(psmb[:], r32(ones_row[0:1, 0:P]), r32(mu[:]),
                                 start=True, stop=True)
                for mt in range(kte):
                    nc.vector.tensor_tensor(src[:, mt, :], src[:, mt, :],
                                            psmb[:], ALU.subtract)
                psvar = pstat.tile([1, sl], F32, tag="stat")
                for mt in range(kte):
                    sq = sqp.tile([P, sl], F32R, tag="sq")
                    nc.vector.tensor_tensor(sq[:], src[:, mt, :],
                                            src[:, mt, :], ALU.mult)
                    nc.tensor.matmul(psvar[:], r32(ones_col[:]), r32(sq[:]),
                                     start=(mt == 0), stop=(mt == kte - 1))
                sstd = statp.tile([1, sl], F32R, tag="stat_sb")
                nc.scalar.activation(sstd[:], psvar[:], AF.Sqrt,
                                     bias=eps_t[0:1, 0:1], scale=1.0 / E)
                rstd = statp.tile([1, sl], F32R, tag="stat_sb")
                nc.vector.reciprocal(rstd[:], sstd[:])
                psrb = psp.tile([P, sl], F32, tag="ps")
                nc.tensor.matmul(psrb[:], r32(ones_row[0:1, 0:P]), r32(rstd[:]),
                                 start=True, stop=True)
                for mt in range(kte):
                    tmp = sqp.tile([P, sl], F32R, tag="sq")
                    nc.vector.tensor_tensor(tmp[:], src[:, mt, :], psrb[:],
                                            ALU.mult)
                    nc.vector.tensor_scalar(tmp[:], tmp[:],
                                            g_c[:, mt:mt + 1], b_c[:, mt:mt + 1],
                                            ALU.mult, ALU.add)
                    nc.vector.tensor_tensor(dst[:, mt, :], tmp[:],
                                            res[:, mt, :], ALU.add)

            with tc.tile_pool(name="qh", bufs=1) as qhp:
                qT = qhp.tile([P, kte, sl], ISLAND, tag="qT")
                heads = qhp.tile([P, kte, sl], F32R, tag="heads")

                # ---------------- phase 1: QKV + AllGather ----------------
                with (
                    tc.tile_pool(name="wcol", bufs=3) as wcp,
                    tc.tile_pool(name="wv_p", bufs=2) as wvp,
                    tc.tile_pool(name="kvstg", bufs=3) as stgp,
                    tc.tile_pool(name="rows1", bufs=1) as rp1,
                    tc.tile_pool(name="ps_qkv", bufs=3, space="PSUM") as psq,
                ):
                    bq_r = row_tile(rp1, bq_d, E, "bq")
                    bk_r = row_tile(rp1, bk_d, E, "bk")
                    bv_r = row_tile(rp1, bv_d, E, "bv")

                    def wcol_chunk(wd, mt):
                        w = wcp.tile([P, kte, P], BF16, tag="wcol")
                        nc.sync.dma_start(
                            w[:], wd[:, mt * P:(mt + 1) * P]
                            .rearrange("(kt p) m -> p kt m", p=P))
                        return w

                    # k^T -> ck_in (bf16) -> AllGather
                    for mt in range(kte):
                        w = wcol_chunk(wk_d, mt)
                        ps = psq.tile([P, sl], F32, tag="ps")
                        for kt in range(kte):
                            nc.tensor.matmul(ps[:], w[:, kt, :],
                                             xtb[:, kt, :],
                                             start=(kt == 0), stop=False)
                        nc.tensor.matmul(ps[:], bk_r[0:1, mt * P:(mt + 1) * P],
                                         ones_row[0:1, 0:sl],
                                         start=False, stop=True)
                        stg = stgp.tile([P, sl], ISLAND, tag="kv_stage")
                        nc.vector.tensor_copy(stg[:], ps[:])
                        nc.sync.dma_start(ck_in[mt * P:(mt + 1) * P, :], stg[:])
                    nc.gpsimd.collective_compute(
                        "AllGather", ALU.bypass, replica_groups=rg,
                        ins=[ck_in.opt()], outs=[ck_out.opt()])

                    # v -> cv_in (bf16) -> AllGather
                    for nt in range(nte):
                        wv = wvp.tile([P, kte, 512], BF16, tag="wv")
                        nc.sync.dma_start(
                            wv[:], wv_d[:, nt * 512:(nt + 1) * 512]
                            .rearrange("(kt p) m -> p kt m", p=P))
                        for st in range(stl):
                            ps = psq.tile([P, 512], F32, tag="ps")
                            for kt in range(kte):
                                nc.tensor.matmul(
                                    ps[:], xtb[:, kt, st * P:(st + 1) * P],
                                    wv[:, kt, :],
                                    start=(kt == 0), stop=False)
                            nc.tensor.matmul(ps[:], ones_row[0:1, 0:P],
                                             bv_r[0:1, nt * 512:(nt + 1) * 512],
                                             start=False, stop=True)
                            stg = stgp.tile([P, 512], ISLAND, tag="kv_stage")
                            nc.vector.tensor_copy(stg[:], ps[:])
                            nc.sync.dma_start(
                                cv_in[st * P:(st + 1) * P,
                                      nt * 512:(nt + 1) * 512], stg[:])
                    nc.gpsimd.collective_compute(
                        "AllGather", ALU.bypass, replica_groups=rg,
                        ins=[cv_in.opt()], outs=[cv_out.opt()])

                    # q^T -> sbuf bf16 (overlaps with the AllGathers)
                    for mt in range(kte):
                        w = wcol_chunk(wq_d, mt)
                        ps = psq.tile([P, sl], F32, tag="ps")
                        for kt in range(kte):
                            nc.tensor.matmul(ps[:], w[:, kt, :],
                                             xtb[:, kt, :],
                                             start=(kt == 0), stop=False)
                        nc.tensor.matmul(ps[:], bq_r[0:1, mt * P:(mt + 1) * P],
                                         ones_row[0:1, 0:sl],
                                         start=False, stop=True)
                        nc.vector.tensor_copy(qT[:, mt, :], ps[:])

                # ---------------- phase 2: attention (bf16 island) ---------
                with (
                    tc.tile_pool(name="attn", bufs=2) as atp,
                    tc.tile_pool(name="vau", bufs=4) as vap,
                    tc.tile_pool(name="exp_p", bufs=4) as exq,
                    tc.tile_pool(name="ps_s", bufs=2, space="PSUM") as pss_p,
                    tc.tile_pool(name="ps_o", bufs=3, space="PSUM") as pso_p,
                ):
                    for hp in range(H // 2):
                        hA, hB = 2 * hp, 2 * hp + 1
                        # k^T head pair: rows 0:64 head A, 64:128 head B
                        kth = atp.tile([P, s_total], ISLAND, tag="kth")
                        for r in range(n_cores):
                            for sub, h in ((0, hA), (1, hB)):
                                nc.sync.dma_start(
                                    kth[sub * DK:(sub + 1) * DK,
                                        r * sl:(r + 1) * sl],
                                    ck_out[r * E + h * DK:
                                           r * E + (h + 1) * DK, :])
                        vaugs = []
                        for h in (hA, hB):
                            va = vap.tile([P, skt, DK + 1], ISLAND, tag="vaug")
                            nc.sync.dma_start(
                                va[:, :, 0:DK],
                                cv_out[:, h * DK:(h + 1) * DK]
                                .rearrange("(t p) d -> p t d", p=P))
                            nc.vector.tensor_copy(
                                va[:, :, DK:DK + 1],
                                ones_f32[:, 0:skt].unsqueeze(2))
                            vaugs.append(va)
                        vaA, vaB = vaugs

                        psoA = pso_p.tile([DK + 1, sl], F32, tag="pso")
                        psoB = pso_p.tile([DK + 1, sl], F32, tag="pso")
                        qA = qT[0:DK, hp, :]
                        qB = qT[DK:2 * DK, hp, :]
                        ex_prev = None
                        for kt in range(skt):
                            # the two scores matmuls pack into PE row groups
                            # 0-63 / 64-127 and run concurrently
                            pss = pss_p.tile([P, 2, sl], F32, tag="pss")
                            nc.tensor.matmul(pss[:, 0, :],
                                             kth[0:DK, kt * P:(kt + 1) * P],
                                             qA, start=True, stop=True)
                            nc.tensor.matmul(pss[:, 1, :],
                                             kth[DK:2 * DK, kt * P:(kt + 1) * P],
                                             qB, start=True, stop=True)
                            ex = exq.tile([P, 2, sl], ISLAND, tag="ex")
                            nc.scalar.activation(ex[:], pss[:], AF.Exp,
                                                 scale=0.125)
                            if PIPELINE:
                                if ex_prev is not None:
                                    ktp = kt - 1
                                    nc.tensor.matmul(
                                        psoA[:], vaA[:, ktp, :],
                                        ex_prev[:, 0, :],
                                        start=(ktp == 0), stop=False,
                                        skip_group_check=True)
                                    nc.tensor.matmul(
                                        psoB[:], vaB[:, ktp, :],
                                        ex_prev[:, 1, :],
                                        start=(ktp == 0), stop=False,
                                        skip_group_check=True)
                                ex_prev = ex
                            else:
                                nc.tensor.matmul(
                                    psoA[:], vaA[:, kt, :], ex[:, 0, :],
                                    start=(kt == 0), stop=(kt == skt - 1),
                                    skip_group_check=True)
                                nc.tensor.matmul(
                                    psoB[:], vaB[:, kt, :], ex[:, 1, :],
                                    start=(kt == 0), stop=(kt == skt - 1),
                                    skip_group_check=True)
                        if PIPELINE:
                            ktp = skt - 1
                            nc.tensor.matmul(psoA[:], vaA[:, ktp, :],
                                             ex_prev[:, 0, :],
                                             start=False, stop=True,
                                             skip_group_check=True)
                            nc.tensor.matmul(psoB[:], vaB[:, ktp, :],
                                             ex_prev[:, 1, :],
                                             start=False, stop=True,
                                             skip_group_check=True)

                        # stash unnormalized heads + 1/denominator; the
                        # normalization happens in the out-proj phase where
                        # PSUM banks are free again
                        for sub, pso in ((0, psoA), (1, psoB)):
                            off = sub * DK
                            h = 2 * hp + sub
                            nc.vector.reciprocal(recips[0:1, h, :],
                                                 pso[DK:DK + 1, :])
                            nc.vector.tensor_copy(heads[off:off + DK, hp, :],
                                                  pso[0:DK, :])

                # ---------------- phase 3: out-proj + LN1 ------------------
                with (
                    tc.tile_pool(name="wo_p", bufs=3) as wop,
                    tc.tile_pool(name="rows3", bufs=1) as rp3,
                    tc.tile_pool(name="z_p", bufs=1) as zp,
                    tc.tile_pool(name="sq3", bufs=2) as sq3,
                    tc.tile_pool(name="ps_m3", bufs=3, space="PSUM") as psm3,
                    tc.tile_pool(name="ps_st3", bufs=2, space="PSUM") as pst3,
                ):
                    bo_r = row_tile(rp3, bo_d, E, "bo")
                    zT = zp.tile([P, kte, sl], F32R, tag="zT")
                    for h in range(H):
                        off = (h % 2) * DK
                        psb = pst3.tile([DK, sl], F32, tag="stat")
                        nc.tensor.matmul(psb[:], ones_row[0:1, 0:DK],
                                         recips[0:1, h, :],
                                         start=True, stop=True)
                        nc.vector.tensor_tensor(heads[off:off + DK, h // 2, :],
                                                heads[off:off + DK, h // 2, :],
                                                psb[:], ALU.mult)
                    for mt in range(kte):
                        w = wop.tile([P, kte, P], F32R, tag="wo")
                        nc.sync.dma_start(
                            w[:], wo_d[:, mt * P:(mt + 1) * P]
                            .rearrange("(kt p) m -> p kt m", p=P))
                        ps = psm3.tile([P, sl], F32, tag="ps")
                        for kt in range(kte):
                            nc.tensor.matmul(ps[:], w[:, kt, :],
                                             heads[:, kt, :],
                                             start=(kt == 0), stop=False)
                        nc.tensor.matmul(ps[:], bo_r[0:1, mt * P:(mt + 1) * P],
                                         ones_row[0:1, 0:sl],
                                         start=False, stop=True)
                        nc.vector.tensor_copy(zT[:, mt, :], ps[:])
                    layer_norm(zT, xt, g1_c, b1_c, y1T, psm3, pst3, sq3)

            # ---------------- phases 4-6: FF + LN2 ----------------
            with (
                tc.tile_pool(name="ff", bufs=1) as ffp,
                tc.tile_pool(name="wf1_p", bufs=4) as wf1p,
                tc.tile_pool(name="wf2_p", bufs=3) as wf2p,
                tc.tile_pool(name="sq4", bufs=2) as sq4,
                tc.tile_pool(name="ps_m4", bufs=3, space="PSUM") as psm4,
                tc.tile_pool(name="ps_st4", bufs=2, space="PSUM") as pst4,
            ):
                hT = ffp.tile([P, ffe, sl], BF16, tag="hT")
                ffT = ffp.tile([P, kte, sl], F32R, tag="ffT")
                y1b = ffp.tile([P, kte, sl], BF16, tag="y1b")
                for mt in range(kte):
                    nc.vector.tensor_copy(y1b[:, mt, :], y1T[:, mt, :])
                bf1_r = row_tile(ffp, bf1_d, FF, "bf1")
                bf2_r = row_tile(ffp, bf2_d, E, "bf2")
                for mt in range(ffe):
                    wt = wf1p.tile([P, kte, P], BF16, tag="wf1")
                    nc.sync.dma_start(
                        wt[:], wf1_d[:, mt * P:(mt + 1) * P]
                        .rearrange("(kt p) m -> p kt m", p=P))
                    ps = psm4.tile([P, sl], F32, tag="ps")
                    for kt in range(kte):
                        nc.tensor.matmul(ps[:], wt[:, kt, :],
                                         y1b[:, kt, :],
                                         start=(kt == 0), stop=False)
                    nc.tensor.matmul(ps[:], bf1_r[0:1, mt * P:(mt + 1) * P],
                                     ones_row[0:1, 0:sl],
                                     start=False, stop=True)
                    nc.vector.tensor_scalar_max(hT[:, mt, :], ps[:], 0.0)
                kg = 8  # kt-group size for streaming W_ff2
                for mt in range(kte):
                    ps = psm4.tile([P, sl], F32, tag="ps")
                    for g in range(ffe // kg):
                        wt2 = wf2p.tile([P, kg, P], BF16, tag="wf2")
                        nc.sync.dma_start(
                            wt2[:], wf2_d[g * kg * P:(g + 1) * kg * P,
                                          mt * P:(mt + 1) * P]
                            .rearrange("(kt p) m -> p kt m", p=P))
                        for j in range(kg):
                            kt = g * kg + j
                            nc.tensor.matmul(ps[:], wt2[:, j, :],
                                             hT[:, kt, :],
                                             start=(kt == 0), stop=False)
                    nc.tensor.matmul(ps[:], bf2_r[0:1, mt * P:(mt + 1) * P],
                                     ones_row[0:1, 0:sl],
                                     start=False, stop=True)
                    nc.vector.tensor_copy(ffT[:, mt, :], ps[:])
                layer_norm(ffT, y1T, g2_c, b2_c, ffT, psm4, pst4, sq4)
                for mt in range(kte):
                    nc.sync.dma_start(outT_d[mt * P:(mt + 1) * P, :],
                                      ffT[:, mt, :])

    nc.compile()
    return nc


_CACHE = {}


def kernel(**inputs):
    global LAST_RESULT
    inp = {k: np.ascontiguousarray(np.asarray(v, dtype=np.float32))
           for k, v in inputs.items()}
    x = inp['encoder_input']
    s_total = x.shape[0]
    n_cores = 8
    sl = s_total // n_cores

    key = (s_total, n_cores)
    if key not in _CACHE:
        _CACHE[key] = build_nc(s_total=s_total, n_cores=n_cores)
    nc = _CACHE[key]

    xT = np.ascontiguousarray(x.T)
    xTb = xT.astype(ml_dtypes.bfloat16)
    bf = lambda a: np.ascontiguousarray(a.astype(ml_dtypes.bfloat16))
    common = {n: inp[n] for n in
              ("Wo", "bq", "bk", "bv", "bo", "b_ff1", "b_ff2",
               "g1", "b1", "g2", "b2")}
    common.update({n: bf(inp[n]) for n in ("Wq", "Wk", "Wv", "W_ff1", "W_ff2")})
    in_maps = [{"xT": np.ascontiguousarray(xT[:, r * sl:(r + 1) * sl]),
                "xTb": np.ascontiguousarray(xTb[:, r * sl:(r + 1) * sl]),
                **common}
               for r in range(n_cores)]

    res = run_bass_kernel_spmd(nc, in_maps, list(range(n_cores)),
                               trace=TRACE, **TRACE_KWARGS)
    LAST_RESULT = res
    out = np.concatenate([res.results[r]["outT"] for r in range(n_cores)],
                         axis=1).T
    return np.ascontiguousarray(out)


# revision 33
# speedup vs baseline: 1.0580x; 1.0206x over previous
# Trainium2 Bass kernel: transformer encoder block, 8-way sequence-parallel.
#
# Sharding: each of the 8 cores owns S/8 = 512 sequence rows (queries). Each
# core computes q^T / k^T / v for its own rows, the k^T and v shards are
# AllGathered (in bf16 to halve wire bytes) so every core sees the full
# keys/values, then each core runs full attention + out-proj + LN + FF + LN
# for its own 512 rows. The host only transposes/slices inputs and
# concatenates/transposes the output.
#
# On-chip data layout is "transposed world": activations live as [E, s]
# tiles (embedding on the partition axis) so every matmul uses the natural
# weight matrix as the stationary lhsT operand and chains without any
# activation transposes. Softmax runs on scores^T [s_k, s_q] tiles: exp on
# the scalar engine, and both the PV product and the softmax denominator
# come from one PE accumulation using a ones-augmented V ([v | 1], M=65).
# Attention processes head PAIRS so the two scores matmuls (K=64, base
# partitions 0/64) pack into disjoint PE row groups and run concurrently,
# and the PV matmuls of step kt-1 are emitted after the scores of step kt
# so the PE never stalls on the scalar engine's exp.
# Projection/FF matmuls use float32r (fp32 bits, 1 cycle/row at N>=256);
# the attention island (q/k/v/exp) runs in bf16.
import sys

if '/opt/trn_rl_repo' not in sys.path:
    sys.path.insert(0, '/opt/trn_rl_repo')

import numpy as np
import ml_dtypes

import concourse.bacc as bacc
import concourse.tile as tile
import concourse.mybir as mybir
from concourse.bass_utils import run_bass_kernel_spmd

F32 = mybir.dt.float32
F32R = mybir.dt.float32r
BF16 = mybir.dt.bfloat16
ISLAND = BF16  # attention-island dtype
PIPELINE = True  # software-pipeline PV behind next scores
AF = mybir.ActivationFunctionType
ALU = mybir.AluOpType

E, H, DK, FF = 1024, 16, 64, 4096
EPS = 1e-5
P = 128

# Knobs test.py can flip.
TRACE = False
TRACE_KWARGS = {}
LAST_RESULT = None


def r32(ap):
    return ap.bitcast(F32R)


def build_nc(s_total=4096, n_cores=8):
    sl = s_total // n_cores      # local sequence rows per core
    kte = E // P                 # contraction tiles over E
    skt = s_total // P           # key tiles over full sequence
    ffe = FF // P                # tiles over FF dim
    nte = E // 512               # 512-wide free tiles over E
    stl = sl // P                # local sequence tiles
    assert sl <= 512

    nc = bacc.Bacc("TRN2", target_bir_lowering=False, debug=False,
                   num_devices=n_cores)

    def din(name, shape, dt=F32R):
        return nc.dram_tensor(name, shape, dt, kind="ExternalInput").ap()

    xT_d = din("xT", [E, sl])
    xTb_d = din("xTb", [E, sl], BF16)
    wq_d, wk_d, wv_d = (din(n, [E, E], BF16) for n in ("Wq", "Wk", "Wv"))
    wo_d = din("Wo", [E, E])
    wf1_d = din("W_ff1", [E, FF], BF16)
    wf2_d = din("W_ff2", [FF, E], BF16)
    bq_d, bk_d, bv_d, bo_d = (din(n, [E]) for n in ("bq", "bk", "bv", "bo"))
    bf1_d = din("b_ff1", [FF])
    bf2_d = din("b_ff2", [E])
    g1_d, b1_d, g2_d, b2_d = (din(n, [E], F32) for n in ("g1", "b1", "g2", "b2"))
    outT_d = nc.dram_tensor("outT", [E, sl], F32R, kind="ExternalOutput").ap()

    ck_in = nc.dram_tensor("ck_in", [E, sl], ISLAND).ap()
    cv_in = nc.dram_tensor("cv_in", [sl, E], ISLAND).ap()
    ck_out = nc.dram_tensor("ck_out", [n_cores * E, sl], ISLAND,
                            addr_space="Shared").ap()
    cv_out = nc.dram_tensor("cv_out", [s_total, E], ISLAND,
                            addr_space="Shared").ap()
    rg = [list(range(n_cores))]

    with tile.TileContext(nc) as tc, \
            nc.allow_low_precision(reason="fp32r/bf16 matmul pipeline"):
        with (
            tc.tile_pool(name="const", bufs=1) as cp,
            tc.tile_pool(name="acts", bufs=1) as ap_,
            tc.tile_pool(name="stat_sb", bufs=4) as statp,
        ):
            # ---- constants ----
            xt = cp.tile([P, kte, sl], F32R, tag="xt")
            nc.sync.dma_start(xt[:], xT_d.rearrange("(kt p) s -> p kt s", p=P))

            # f32r/bf16 constants must be produced by a rounding engine op,
            # not memset: memset an f32 scratch, then DVE-copy (casts).
            ones_f32 = cp.tile([P, 512], F32, tag="ones_f32")
            nc.vector.memset(ones_f32[:], 1.0)
            ones_row = cp.tile([1, 512], F32R, tag="ones_row")
            nc.vector.tensor_copy(ones_row[:], ones_f32[0:1, 0:512])
            ones_col = cp.tile([P, 1], F32R, tag="ones_col")
            nc.vector.tensor_copy(ones_col[:], ones_f32[:, 0:1])
            eps_t = cp.tile([1, 1], F32, tag="eps")
            nc.vector.memset(eps_t[:], EPS)

            def row_tile(pool, d, n, tag):
                t = pool.tile([1, n], F32R, tag=tag)
                nc.sync.dma_start(t[:], d.unsqueeze(0))
                return t

            def col_tile(d, n, tag):
                t = cp.tile([P, n // P], F32, tag=tag)
                nc.sync.dma_start(t[:], d.rearrange("(t p) -> p t", p=P))
                return t

            g1_c = col_tile(g1_d, E, "g1")
            b1_c = col_tile(b1_d, E, "b1")
            g2_c = col_tile(g2_d, E, "g2")
            b2_c = col_tile(b2_d, E, "b2")

            # persistent activations
            y1T = ap_.tile([P, kte, sl], F32R, tag="y1T")
            recips = ap_.tile([1, H, sl], F32R, tag="recips")
            dstash = ap_.tile([1, H, sl], F32, tag="dstash")

            def layer_norm(src, res, g_c, b_c, dst, psp, pstat, sqp):
                """dst = res + LN(src) * g + b, over the partition (E) axis.

                Uses var = E[x^2] - mu^2 so the x and x^2 column-sums
                accumulate concurrently (squares on the scalar engine).
                """
                psmu = pstat.tile([1, sl], F32, tag="stat")
                psvar = pstat.tile([1, sl], F32, tag="stat")
                for mt in range(kte):
                    nc.tensor.matmul(psmu[:], r32(ones_col[:]),
                                     r32(src[:, mt, :]),
                                     start=(mt == 0), stop=(mt == kte - 1))
                for mt in range(kte):
                    sq = sqp.tile([P, sl], F32R, tag="sq")
                    nc.scalar.activation(sq[:], src[:, mt, :], AF.Square)
                    nc.tensor.matmul(psvar[:], r32(ones_col[:]), r32(sq[:]),
                                     start=(mt == 0), stop=(mt == kte - 1))
                mu = statp.tile([1, sl], F32R, tag="stat_sb")
                nc.scalar.mul(mu[:], psmu[:], 1.0 / E)
                musq = statp.tile([1, sl], F32, tag="stat_sb")
                nc.vector.tensor_tensor(musq[:], mu[:], mu[:], ALU.mult)
                var = statp.tile([1, sl], F32, tag="stat_sb")
                nc.vector.tensor_scalar(var[:], psvar[:], 1.0 / E, None,
                                        ALU.mult)
                nc.vector.tensor_tensor(var[:], var[:], musq[:], ALU.subtract)
                sstd = statp.tile([1, sl], F32R, tag="stat_sb")
                nc.scalar.activation(sstd[:], var[:], AF.Sqrt,
                                     bias=eps_t[0:1, 0:1])
                rstd = statp.tile([1, sl], F32R, tag="stat_sb")
                nc.vector.reciprocal(rstd[:], sstd[:])
                psmb = psp.tile([P, sl], F32, tag="ps")
                nc.tensor.matmul(psmb[:], r32(ones_row[0:1, 0:P]), r32(mu[:]),
                                 start=True, stop=True)
                psrb = psp.tile([P, sl], F32, tag="ps")
                nc.tensor.matmul(psrb[:], r32(ones_row[0:1, 0:P]), r32(rstd[:]),
                                 start=True, stop=True)
                for mt in range(kte):
                    tmp = sqp.tile([P, sl], F32R, tag="sq")
                    nc.vector.tensor_tensor(tmp[:], src[:, mt, :], psmb[:],
                                            ALU.subtract)
                    nc.vector.scalar_tensor_tensor(tmp[:], tmp[:],
                                                   g_c[:, mt:mt + 1], psrb[:],
                                                   ALU.mult, ALU.mult)
                    nc.vector.tensor_scalar(tmp[:], tmp[:],
                                            b_c[:, mt:mt + 1], None, ALU.add)
                    nc.vector.tensor_tensor(dst[:, mt, :], tmp[:],
                                            res[:, mt, :], ALU.add)

            with tc.tile_pool(name="qh", bufs=1) as qhp:
                qT = qhp.tile([P, kte, sl], ISLAND, tag="qT")
                heads = qhp.tile([P, kte, sl], F32R, tag="heads")

                # ---------------- phase 1: QKV + AllGather ----------------
                with (
                    tc.tile_pool(name="wcol", bufs=3) as wcp,
                    tc.tile_pool(name="wv_p", bufs=2) as wvp,
                    tc.tile_pool(name="kvstg", bufs=3) as stgp,
                    tc.tile_pool(name="rows1", bufs=1) as rp1,
                    tc.tile_pool(name="ps_qkv", bufs=3, space="PSUM") as psq,
                ):
                    xtb = wvp.tile([P, kte, sl], BF16, tag="xtb")
                    nc.sync.dma_start(
                        xtb[:], xTb_d.rearrange("(kt p) s -> p kt s", p=P))
                    bq_r = row_tile(rp1, bq_d, E, "bq")
                    bk_r = row_tile(rp1, bk_d, E, "bk")
                    bv_r = row_tile(rp1, bv_d, E, "bv")

                    def wcol_chunk(wd, mt):
                        w = wcp.tile([P, kte, P], BF16, tag="wcol")
                        nc.sync.dma_start(
                            w[:], wd[:, mt * P:(mt + 1) * P]
                            .rearrange("(kt p) m -> p kt m", p=P))
                        return w

                    # k^T -> ck_in (bf16) -> AllGather
                    for mt in range(kte):
                        w = wcol_chunk(wk_d, mt)
                        ps = psq.tile([P, sl], F32, tag="ps")
                        for kt in range(kte):
                            nc.tensor.matmul(ps[:], w[:, kt, :],
                                             xtb[:, kt, :],
                                             start=(kt == 0), stop=False)
                        nc.tensor.matmul(ps[:], bk_r[0:1, mt * P:(mt + 1) * P],
                                         ones_row[0:1, 0:sl],
                                         start=False, stop=True)
                        stg = stgp.tile([P, sl], ISLAND, tag="kv_stage")
                        nc.vector.tensor_copy(stg[:], ps[:])
                        nc.sync.dma_start(ck_in[mt * P:(mt + 1) * P, :], stg[:])
                    nc.gpsimd.collective_compute(
                        "AllGather", ALU.bypass, replica_groups=rg,
                        ins=[ck_in.opt()], outs=[ck_out.opt()])

                    # v -> cv_in (bf16) -> AllGather
                    for nt in range(nte):
                        wv = wvp.tile([P, kte, 512], BF16, tag="wv")
                        nc.sync.dma_start(
                            wv[:], wv_d[:, nt * 512:(nt + 1) * 512]
                            .rearrange("(kt p) m -> p kt m", p=P))
                        for st in range(stl):
                            ps = psq.tile([P, 512], F32, tag="ps")
                            for kt in range(kte):
                                nc.tensor.matmul(
                                    ps[:], xtb[:, kt, st * P:(st + 1) * P],
                                    wv[:, kt, :],
                                    start=(kt == 0), stop=False)
                            nc.tensor.matmul(ps[:], ones_row[0:1, 0:P],
                                             bv_r[0:1, nt * 512:(nt + 1) * 512],
                                             start=False, stop=True)
                            stg = stgp.tile([P, 512], ISLAND, tag="kv_stage")
                            nc.vector.tensor_copy(stg[:], ps[:])
                            nc.sync.dma_start(
                                cv_in[st * P:(st + 1) * P,
                                      nt * 512:(nt + 1) * 512], stg[:])
                    nc.gpsimd.collective_compute(
                        "AllGather", ALU.bypass, replica_groups=rg,
                        ins=[cv_in.opt()], outs=[cv_out.opt()])

                    # q^T -> sbuf bf16 (overlaps with the AllGathers)
                    for mt in range(kte):
                        w = wcol_chunk(wq_d, mt)
                        ps = psq.tile([P, sl], F32, tag="ps")
                        for kt in range(kte):
                            nc.tensor.matmul(ps[:], w[:, kt, :],
                                             xtb[:, kt, :],
                                             start=(kt == 0), stop=False)
                        nc.tensor.matmul(ps[:], bq_r[0:1, mt * P:(mt + 1) * P],
                                         ones_row[0:1, 0:sl],
                                         start=False, stop=True)
                        nc.vector.tensor_copy(qT[:, mt, :], ps[:])

                # ---------------- phase 2: attention (bf16 island) ---------
                with (
                    tc.tile_pool(name="attn", bufs=2) as atp,
                    tc.tile_pool(name="vau", bufs=4) as vap,
                    tc.tile_pool(name="exp_p", bufs=4) as exq,
                    tc.tile_pool(name="ps_s", bufs=2, space="PSUM") as pss_p,
                    tc.tile_pool(name="ps_o", bufs=3, space="PSUM") as pso_p,
                ):
                    for hp in range(H // 2):
                        hA, hB = 2 * hp, 2 * hp + 1
                        # k^T head pair: rows 0:64 head A, 64:128 head B
                        kth = atp.tile([P, s_total], ISLAND, tag="kth")
                        for r in range(n_cores):
                            for sub, h in ((0, hA), (1, hB)):
                                nc.sync.dma_start(
                                    kth[sub * DK:(sub + 1) * DK,
                                        r * sl:(r + 1) * sl],
                                    ck_out[r * E + h * DK:
                                           r * E + (h + 1) * DK, :])
                        vaugs = []
                        for h in (hA, hB):
                            va = vap.tile([P, skt, DK + 1], ISLAND, tag="vaug")
                            nc.sync.dma_start(
                                va[:, :, 0:DK],
                                cv_out[:, h * DK:(h + 1) * DK]
                                .rearrange("(t p) d -> p t d", p=P))
                            nc.vector.tensor_copy(
                                va[:, :, DK:DK + 1],
                                ones_f32[:, 0:skt].unsqueeze(2))
                            vaugs.append(va)
                        vaA, vaB = vaugs

                        psoA = pso_p.tile([DK + 1, sl], F32, tag="pso")
                        psoB = pso_p.tile([DK + 1, sl], F32, tag="pso")
                        qA = qT[0:DK, hp, :]
                        qB = qT[DK:2 * DK, hp, :]
                        ex_prev = None
                        for kt in range(skt):
                            # the two scores matmuls pack into PE row groups
                            # 0-63 / 64-127 and run concurrently
                            pss = pss_p.tile([P, 2, sl], F32, tag="pss")
                            nc.tensor.matmul(pss[:, 0, :],
                                             kth[0:DK, kt * P:(kt + 1) * P],
                                             qA, start=True, stop=True)
                            nc.tensor.matmul(pss[:, 1, :],
                                             kth[DK:2 * DK, kt * P:(kt + 1) * P],
                                             qB, start=True, stop=True)
                            ex = exq.tile([P, 2, sl], ISLAND, tag="ex")
                            nc.scalar.activation(ex[:], pss[:], AF.Exp,
                                                 scale=0.125)
                            if PIPELINE:
                                if ex_prev is not None:
                                    ktp = kt - 1
                                    nc.tensor.matmul(
                                        psoA[:], vaA[:, ktp, :],
                                        ex_prev[:, 0, :],
                                        start=(ktp == 0), stop=False,
                                        skip_group_check=True)
                                    nc.tensor.matmul(
                                        psoB[:], vaB[:, ktp, :],
                                        ex_prev[:, 1, :],
                                        start=(ktp == 0), stop=False,
                                        skip_group_check=True)
                                ex_prev = ex
                            else:
                                nc.tensor.matmul(
                                    psoA[:], vaA[:, kt, :], ex[:, 0, :],
                                    start=(kt == 0), stop=(kt == skt - 1),
                                    skip_group_check=True)
                                nc.tensor.matmul(
                                    psoB[:], vaB[:, kt, :], ex[:, 1, :],
                                    start=(kt == 0), stop=(kt == skt - 1),
                                    skip_group_check=True)
                        if PIPELINE:
                            ktp = skt - 1
                            nc.tensor.matmul(psoA[:], vaA[:, ktp, :],
                                             ex_prev[:, 0, :],
                                             start=False, stop=True,
                                             skip_group_check=True)
                            nc.tensor.matmul(psoB[:], vaB[:, ktp, :],
                                             ex_prev[:, 1, :],
                                             start=False, stop=True,
                                             skip_group_check=True)

                        # stash unnormalized heads + 1/denominator; the
                        # normalization happens in the out-proj phase where
                        # PSUM banks are free again
                        for sub, pso in ((0, psoA), (1, psoB)):
                            off = sub * DK
                            h = 2 * hp + sub
                            nc.vector.tensor_copy(dstash[0:1, h, :],
                                                  pso[DK:DK + 1, :])
                            nc.vector.tensor_copy(heads[off:off + DK, hp, :],
                                                  pso[0:DK, :])

                    # batched reciprocal of all 16 softmax denominators:
                    # gather [1,H,sl] -> [H,sl] via DMA, one DVE reciprocal,
                    # scatter back to row layout for the bcast matmuls
                    dall = atp.tile([H, sl], F32, tag="dall")
                    nc.sync.dma_start(dall[:], dstash[0:1, :, :])
                    rall = atp.tile([H, sl], F32R, tag="rall")
                    nc.vector.reciprocal(rall[:], dall[:])
                    nc.sync.dma_start(recips[0:1, :, :], rall[:])

                # ---------------- phase 3: out-proj + LN1 ------------------
                with (
                    tc.tile_pool(name="wo_p", bufs=3) as wop,
                    tc.tile_pool(name="rows3", bufs=1) as rp3,
                    tc.tile_pool(name="z_p", bufs=1) as zp,
                    tc.tile_pool(name="sq3", bufs=2) as sq3,
                    tc.tile_pool(name="ps_m3", bufs=3, space="PSUM") as psm3,
                    tc.tile_pool(name="ps_st3", bufs=2, space="PSUM") as pst3,
                ):
                    bo_r = row_tile(rp3, bo_d, E, "bo")
                    zT = zp.tile([P, kte, sl], F32R, tag="zT")
                    for h in range(H):
                        off = (h % 2) * DK
                        psb = pst3.tile([DK, sl], F32, tag="stat")
                        nc.tensor.matmul(psb[:], ones_row[0:1, 0:DK],
                                         recips[0:1, h, :],
                                         start=True, stop=True)
                        nc.vector.tensor_tensor(heads[off:off + DK, h // 2, :],
                                                heads[off:off + DK, h // 2, :],
                                                psb[:], ALU.mult)
                    for mt in range(kte):
                        w = wop.tile([P, kte, P], F32R, tag="wo")
                        nc.sync.dma_start(
                            w[:], wo_d[:, mt * P:(mt + 1) * P]
                            .rearrange("(kt p) m -> p kt m", p=P))
                        ps = psm3.tile([P, sl], F32, tag="ps")
                        for kt in range(kte):
                            nc.tensor.matmul(ps[:], w[:, kt, :],
                                             heads[:, kt, :],
                                             start=(kt == 0), stop=False)
                        nc.tensor.matmul(ps[:], bo_r[0:1, mt * P:(mt + 1) * P],
                                         ones_row[0:1, 0:sl],
                                         start=False, stop=True)
                        nc.vector.tensor_copy(zT[:, mt, :], ps[:])
                    layer_norm(zT, xt, g1_c, b1_c, y1T, psm3, pst3, sq3)

            # ---------------- phases 4-6: FF + LN2 ----------------
            with (
                tc.tile_pool(name="ff", bufs=1) as ffp,
                tc.tile_pool(name="wf1_p", bufs=3) as wf1p,
                tc.tile_pool(name="wf2_p", bufs=3) as wf2p,
                tc.tile_pool(name="sq4", bufs=2) as sq4,
                tc.tile_pool(name="ps_m4", bufs=3, space="PSUM") as psm4,
                tc.tile_pool(name="ps_st4", bufs=2, space="PSUM") as pst4,
            ):
                hT = ffp.tile([P, ffe, sl], BF16, tag="hT")
                ffT = ffp.tile([P, kte, sl], F32R, tag="ffT")
                y1b = ffp.tile([P, kte, sl], BF16, tag="y1b")
                for mt in range(kte):
                    nc.vector.tensor_copy(y1b[:, mt, :], y1T[:, mt, :])
                bf1_r = row_tile(ffp, bf1_d, FF, "bf1")
                bf2_r = row_tile(ffp, bf2_d, E, "bf2")
                for mt in range(ffe):
                    wt = wf1p.tile([P, kte, P], BF16, tag="wf1")
                    nc.sync.dma_start(
                        wt[:], wf1_d[:, mt * P:(mt + 1) * P]
                        .rearrange("(kt p) m -> p kt m", p=P))
                    ps = psm4.tile([P, sl], F32, tag="ps")
                    for kt in range(kte):
                        nc.tensor.matmul(ps[:], wt[:, kt, :],
                                         y1b[:, kt, :],
                                         start=(kt == 0), stop=False)
                    nc.tensor.matmul(ps[:], bf1_r[0:1, mt * P:(mt + 1) * P],
                                     ones_row[0:1, 0:sl],
                                     start=False, stop=True)
                    nc.vector.tensor_scalar_max(hT[:, mt, :], ps[:], 0.0)
                kg = 8  # kt-group size for streaming W_ff2
                for mt in range(kte):
                    ps = psm4.tile([P, sl], F32, tag="ps")
                    for g in range(ffe // kg):
                        wt2 = wf2p.tile([P, kg, P], BF16, tag="wf2")
                        nc.sync.dma_start(
                            wt2[:], wf2_d[g * kg * P:(g + 1) * kg * P,
                                          mt * P:(mt + 1) * P]
                            .rearrange("(kt p) m -> p kt m", p=P))
                        for j in range(kg):
                            kt = g * kg + j
                            nc.tensor.matmul(ps[:], wt2[:, j, :],
                                             hT[:, kt, :],
                                             start=(kt == 0), stop=False)
                    nc.tensor.matmul(ps[:], bf2_r[0:1, mt * P:(mt + 1) * P],
                                     ones_row[0:1, 0:sl],
                                     start=False, stop=True)
                    nc.vector.tensor_copy(ffT[:, mt, :], ps[:])
                layer_norm(ffT, y1T, g2_c, b2_c, ffT, psm4, pst4, sq4)
                for mt in range(kte):
                    nc.sync.dma_start(outT_d[mt * P:(mt + 1) * P, :],
                                      ffT[:, mt, :])

    nc.compile()
    return nc


_CACHE = {}


def kernel(**inputs):
    global LAST_RESULT
    inp = {k: np.ascontiguousarray(np.asarray(v, dtype=np.float32))
           for k, v in inputs.items()}
    x = inp['encoder_input']
    s_total = x.shape[0]
    n_cores = 8
    sl = s_total // n_cores

    key = (s_total, n_cores)
    if key not in _CACHE:
        _CACHE[key] = build_nc(s_total=s_total, n_cores=n_cores)
    nc = _CACHE[key]

    xT = np.ascontiguousarray(x.T)
    xTb = xT.astype(ml_dtypes.bfloat16)
    bf = lambda a: np.ascontiguousarray(a.astype(ml_dtypes.bfloat16))
    common = {n: inp[n] for n in
              ("Wo", "bq", "bk", "bv", "bo", "b_ff1", "b_ff2",
               "g1", "b1", "g2", "b2")}
    common.update({n: bf(inp[n]) for n in ("Wq", "Wk", "Wv", "W_ff1", "W_ff2")})
    in_maps = [{"xT": np.ascontiguousarray(xT[:, r * sl:(r + 1) * sl]),
                "xTb": np.ascontiguousarray(xTb[:, r * sl:(r + 1) * sl]),
                **common}
               for r in range(n_cores)]

    res = run_bass_kernel_spmd(nc, in_maps, list(range(n_cores)),
                               trace=TRACE, **TRACE_KWARGS)
    LAST_RESULT = res
    out = np.concatenate([res.results[r]["outT"] for r in range(n_cores)],
                         axis=1).T
    return np.ascontiguousarray(out)
